# revision 32
# baseline (speedup 1.0000x reference)
"""Trainium2 Bass kernel for AssociativeIncrementalAttention.

Multi-head attention (B=2, S=2048, D=512, H=8, HD=64) with additive
[B,S,S] bias tensors, a concept-equality bias, and key-padding mask.

Sharding: 8 cores, fully data-parallel (no collectives).
  core c -> batch b = c//4, query rows q0 = (c%4)*512 .. q0+512.

v2 design notes (engine-work reduction vs v1):
  - All large DRAM inputs are host-cast to bf16 (halves HBM traffic and
    the startup DMA head; matmuls ran in bf16 anyway).
  - Concept bias (same concept id, both valid, off-diagonal) is computed
    on the PE as a one-hot inner product: cs = onehotK^T @ (0.5*onehotQ)
    over the 64 concept ids, 16 tiny matmuls; replaces ~35us of DVE
    compare work. Diagonal exclusion multiplies cs by (qpos != kpos) on
    the 4 key chunks that contain the diagonal (exact, incl. cid<0).
  - bk is dropped exactly: softmax over k is invariant to the q.bk and
    bq.bk terms of (q+bq).(k+bk); only bq.k survives, so bq stays on Q
    and K needs no bias.  bv/bo ride the output epilogue (cvec) since
    softmax rows sum to 1.
  - Scalar (Act) engine does exp only (plus tiny psq-identity); PSUM->
    SBUF copies are split between DVE/Act/GpSimd to balance busy time.
  - Softmax normalization deferred past attn@V (rowsum rides a ones
    column in V); applied as recip + partition_broadcast + multiply,
    interleaved into the next head's groups.
  - attention_mask / key_padding_mask are all-zero in the target inputs;
    the fast variant skips them (host-checked), a generic variant with
    both is compiled lazily if they are ever nonzero.

Self-contained: hardcodes shapes; host-side prep is layout-only
(slices/transposes) plus dtype casts and tiny metadata encodings
(concept ids -> bf16 sentinel values, position iotas).
"""

import sys

if "/opt/trn_rl_repo" not in sys.path:
    sys.path.insert(0, "/opt/trn_rl_repo")

import numpy as np
import ml_dtypes

import concourse.bass as bass
import concourse.tile as tile
from concourse import bacc, mybir
from concourse import bass_utils

B, S, D, H = 2, 2048, 512, 8
HD = D // H  # 64
N_CORES = 8
QS = 512          # query rows per core
QT = QS // 128    # 4 query tiles per core
DC = D // 128     # 4 contraction chunks
SC512 = S // 512  # 4
SC128 = S // 128  # 16
NC = 64           # number of concept ids
F32 = mybir.dt.float32
F16 = mybir.dt.float16
BF16 = mybir.dt.bfloat16

BF = ml_dtypes.bfloat16

_COMPILED = {}


def _build(with_masks: bool):
    nc = bacc.Bacc("TRN2", target_bir_lowering=False, debug=False,
                   num_devices=N_CORES)

    def din(name, shape, dt=BF16):
        return nc.dram_tensor(name, shape, dt, kind="ExternalInput").ap()

    xT = din("xT", [D, S])            # x[b].T
    xqT = din("xqT", [D, QS])         # x[b, q0:q0+QS].T
    wqT = din("wqT", [D, D])
    wkT = din("wkT", [D, D])
    wvT = din("wvT", [D, D])
    woT = din("woT", [D, D])
    bo = din("bo", [D], F32)
    misc = din("misc", [128, 24], F32)  # kposT | bq cols | bv cols
    F8 = mybir.dt.float8e4
    ipaT = din("ipaT", [S, QS], F8)   # biases ~N(0,0.1): e4m3 quant err
    ascT = din("ascT", [S, QS], F8)   # ~0.004 abs on scores, negligible
    if with_masks:
        mskT = din("mskT", [S, QS])
        kpmT = din("kpmT", [128, SC128], F32)
    cidq = din("cidq", [QS])          # concept ids (neg -> -1 sentinel)
    cidk = din("cidk", [S])           # concept ids (neg -> -2 sentinel)
    qpos = din("qpos", [QS], F16)
    out = nc.dram_tensor("out", [QS, D], BF16, kind="ExternalOutput").ap()

    AL = mybir.AluOpType
    AF = mybir.ActivationFunctionType

    def bcast_ap(src, parts):
        # partition-broadcast read: [[0,parts]] + original free dims
        return bass.AP(tensor=src.tensor, offset=src.offset,
                       ap=[[0, parts]] + list(src.ap))

    # which kc chunks contain diagonal cells depends on q0 (per-core), but
    # the kernel is compiled once for all cores. The diagonal columns are
    # found at runtime by comparing qpos/kpos tiles, so we simply apply
    # the (qpos != kpos) multiply on ALL chunks' pairs... that would cost
    # 16 extra ops; instead exploit that each core's diagonal lies in the
    # 4 chunks q0/128..q0/128+3 -- but q0 is per-core. Trick: qpos/kpos
    # comparison is data-driven, so apply the diag-multiply on all 8
    # pairs only if needed. Cheaper: host passes kpos shifted so that the
    # compare is false outside the diag chunks anyway; the multiply by
    # e in {0,1} is exact everywhere, so applying it everywhere is always
    # correct. We bound cost by applying per-pair (8 pairs) with a
    # [128,1]-scalar compare per chunk (16 ts + 16 tt).
    # -> final choice: apply on all chunks (uniform, data-driven, safe).

    with tile.TileContext(nc) as tc:
        with (
            tc.tile_pool(name="persist", bufs=1) as P,
            tc.tile_pool(name="ipain", bufs=2) as IPA,
            tc.tile_pool(name="ascin", bufs=2) as ASC,
            tc.tile_pool(name="xfall", bufs=2) as XF,
            tc.tile_pool(name="ework", bufs=3) as EW,
            tc.tile_pool(name="praww", bufs=8) as PRW,
            tc.tile_pool(name="pfw", bufs=8) as PW,
            tc.tile_pool(name="rswork", bufs=2) as RW,
            tc.tile_pool(name="osb", bufs=2) as OS,
            tc.tile_pool(name="spp", bufs=3, space="PSUM") as SPP,
            tc.tile_pool(name="ctxp", bufs=2, space="PSUM") as CP,
        ):
            # ---- persistent tiles ----
            kT_sb = P.tile([128, DC, S], BF16, tag="kT")
            qT_sb = P.tile([128, DC, QS], BF16, tag="qT")
            vplus = P.tile([128, SC128, 8 * 65], BF16, tag="vplus")
            vp4 = vplus.rearrange("p s (h c) -> p s h c", c=65)
            ebT = P.tile([128, SC128, QS], BF16, tag="ebT")
            ctxT_sb = P.tile([128, DC, QS], BF16, tag="ctxT")

            # ---- small input loads first: tiny transfers, but they gate
            # the one-hot/combine chain and smalls queue behind big
            # transfers if issued later ----
            misc_sb = P.tile([128, 24], F32, tag="misc")
            nc.sync.dma_start(out=misc_sb, in_=misc)
            kposT_sb = misc_sb[:, 0:16]
            iota_col = misc_sb[0:NC, 0:1]
            cidkb = P.tile([NC, S], BF16, tag="cidkb")
            nc.sync.dma_start(out=cidkb, in_=bcast_ap(cidk, NC))
            cidqb = P.tile([NC, QS], BF16, tag="cidqb")
            nc.sync.dma_start(out=cidqb, in_=bcast_ap(cidq, NC))
            qposb = P.tile([128, QS], F16, tag="qposb")
            nc.sync.dma_start(out=qposb, in_=bcast_ap(qpos, 128))
            w_sb = {}
            w_sb["wq"] = P.tile([128, DC, D], BF16, tag="wq", name="wq")
            nc.sync.dma_start(
                out=w_sb["wq"],
                in_=wqT.rearrange("(c p) s -> p c s", p=128))
            xqT_sb = P.tile([128, DC, QS], BF16, tag="xqT")
            nc.sync.dma_start(
                out=xqT_sb, in_=xqT.rearrange("(c p) s -> p c s", p=128))
            bo_row = P.tile([1, D], F32, tag="bo_row")
            nc.sync.dma_start(out=bo_row, in_=bo.rearrange("(a s) -> a s", a=1))
            if with_masks:
                kpmT_sb = P.tile([128, SC128], F32, tag="kpmT")
                nc.sync.dma_start(out=kpmT_sb, in_=kpmT)

            # ---- big input loads (gpsimd queue; issue order ~ arrival) ----
            def load_w(nm, ap_):
                w_sb[nm] = P.tile([128, DC, D], BF16, tag=nm, name=nm)
                nc.gpsimd.dma_start(
                    out=w_sb[nm], in_=ap_.rearrange("(c p) s -> p c s", p=128))

            load_w("wk", wkT)
            load_w("wv", wvT)
            xT_sb = P.tile([128, DC, S], BF16, tag="xT")

            def load_x_half(i):
                nc.gpsimd.dma_start(
                    out=xT_sb[:, :, i * 1024:(i + 1) * 1024],
                    in_=xT.rearrange("(c p) s -> p c s", p=128)[
                        :, :, i * 1024:(i + 1) * 1024])

            bias_in = {"ipa": [None] * 2, "asc": [None] * 2,
                       "msk": [None] * 2}

            def load_bias_g8(g8):
                specs = [("ipa", ipaT, IPA), ("asc", ascT, ASC)]
                if with_masks:
                    specs.append(("msk", mskT, EW))
                for nm, ap_, pool in specs:
                    t = pool.tile([128, 8, QS], BF16, tag="b" + nm, name=nm)
                    nc.gpsimd.dma_start(
                        out=t,
                        in_=ap_[g8 * 1024:(g8 + 1) * 1024, :].rearrange(
                            "(c p) s -> p c s", p=128))
                    bias_in[nm][g8] = t

            load_x_half(0)
            load_bias_g8(0)
            load_x_half(1)
            load_bias_g8(1)
            load_w("wo", woT)
            wo_sb = w_sb["wo"]

            bv_bf = P.tile([128, DC], BF16, tag="bv_bf")
            nc.vector.tensor_copy(bv_bf, misc_sb[:, 20:24])
            cvec = P.tile([1, D], BF16, tag="cvec")
            ones_row = P.tile([1, 128], BF16, tag="ones_row")
            nc.vector.memset(ones_row, 1.0)
            nc.vector.memset(vp4[:, :, :, 64:65], 1.0)

            # ---- one-hot concept encodings (DVE, tiny) ----
            ohk = P.tile([NC, S], BF16, tag="ohk")
            nc.vector.tensor_scalar(
                out=ohk, in0=cidkb, scalar1=iota_col, scalar2=None,
                op0=AL.is_equal)
            ohq = P.tile([NC, QS], BF16, tag="ohq")
            nc.vector.tensor_scalar(
                out=ohq, in0=cidqb, scalar1=iota_col, scalar2=0.5,
                op0=AL.is_equal, op1=AL.mult)

            # ---- combined bias -> exp, in [k, q] layout, per kc pair ----
            # xfall = cs*(qpos!=kpos) + ipa + asc (+ msk + kpm); eb = exp().
            def combine_pair(p, cs):
                g8 = p // 4
                ipa_t = bias_in["ipa"][g8]
                asc_t = bias_in["asc"][g8]
                jj = (p % 4) * 2
                xf = XF.tile([128, 2, QS], BF16, tag="xf", name="xf")
                for j in range(2):
                    kc = p * 2 + j
                    # xf = (qpos != kpos) * cs, straight from PSUM (the
                    # tile is consumed immediately, so no ring pressure)
                    nc.vector.scalar_tensor_tensor(
                        out=xf[:, j, :], in0=qposb,
                        scalar=kposT_sb[:, kc:kc + 1],
                        in1=cs[:, j, :],
                        op0=AL.not_equal, op1=AL.mult)
                t1 = XF.tile([128, 2, QS], BF16, tag="t1", name="t1")
                nc.vector.tensor_tensor(
                    out=t1, in0=xf, in1=ipa_t[:, jj:jj + 2, :], op=AL.add)
                dst = XF.tile([128, 2, QS], BF16, tag="t2", name="t2")
                nc.vector.tensor_tensor(
                    out=dst, in0=t1, in1=asc_t[:, jj:jj + 2, :], op=AL.add)
                if with_masks:
                    dst2 = XF.tile([128, 2, QS], BF16, tag="t3", name="t3")
                    nc.vector.tensor_tensor(
                        out=dst2, in0=dst, in1=bias_in["msk"][g8][:, jj:jj + 2, :],
                        op=AL.add)
                    kcol = EW.tile([128, 2, QS], BF16, tag="kp", name="kp")
                    for j in range(2):
                        kc = p * 2 + j
                        nc.vector.tensor_scalar(
                            out=kcol[:, j, :], in0=dst2[:, j, :],
                            scalar1=kpmT_sb[:, kc:kc + 1], scalar2=None,
                            op0=AL.add)
                    dst = kcol
                # exp on Act
                nc.scalar.activation(
                    out=ebT[:, p * 2:(p + 1) * 2, :], in_=dst, func=AF.Exp)

            # ---- Q projection (+bq via Act identity) ----
            for ocp in range(2):
                ps = SPP.tile([128, 2, 512], F32, tag="sp", name="psq")
                for i in range(2):
                    oc = ocp * 2 + i
                    for dc in range(DC):
                        nc.tensor.matmul(
                            ps[:, i, :],
                            lhsT=w_sb["wq"][:, dc, oc * 128:(oc + 1) * 128],
                            rhs=xqT_sb[:, dc, :],
                            start=(dc == 0), stop=(dc == DC - 1))
                for i in range(2):
                    oc = ocp * 2 + i
                    nc.scalar.activation(
                        out=qT_sb[:, oc, :], in_=ps[:, i, :],
                        func=AF.Identity, bias=misc_sb[:, 16 + oc:17 + oc])

            # ---- attention main loop ----
            norm_state = {}

            def norm_step(step, hprev, cps_prev):
                ocp_, rbp = hprev // 2, (hprev % 2) * 64
                if step == 0:
                    rs_row = RW.tile([1, QS], F32, tag="rs_row",
                                     name="rs_row")
                    nc.vector.tensor_copy(rs_row, cps_prev[64:65, :])
                    rr = RW.tile([1, QS], F32, tag="rr", name="rr")
                    nc.vector.reciprocal_approx_fast(rr, rs_row)
                    norm_state["rr"] = rr
                elif step == 1:
                    rrb = RW.tile([64, QS], F32, tag="rrb", name="rrb")
                    nc.gpsimd.partition_broadcast(rrb, norm_state["rr"])
                    norm_state["rrb"] = rrb
                else:
                    nc.vector.tensor_tensor(
                        out=ctxT_sb[rbp:rbp + 64, ocp_, :],
                        in0=cps_prev[0:64, :], in1=norm_state["rrb"],
                        op=AL.mult)

            # ---- JIT emitters: K sc-block / V pair produced inside h0's
            # loop right before their first consumer, so no engine queue
            # entry waits on a DMA that lands later than its turn ----
            def emit_k_block(sc):
                for ocp in range(2):
                    ps = SPP.tile([128, 2, 512], F32, tag="sp", name="psk")
                    for i in range(2):
                        oc = ocp * 2 + i
                        for dc in range(DC):
                            nc.tensor.matmul(
                                ps[:, i, :],
                                lhsT=w_sb["wk"][:, dc,
                                                oc * 128:(oc + 1) * 128],
                                rhs=xT_sb[:, dc, sc * 512:(sc + 1) * 512],
                                start=(dc == 0), stop=(dc == DC - 1))
                    dst = kT_sb[:, ocp * 2:(ocp + 1) * 2,
                                sc * 512:(sc + 1) * 512]
                    if ocp == 0:
                        nc.vector.tensor_copy(dst, ps)
                    else:
                        nc.scalar.copy(out=dst, in_=ps)

            def emit_v_pair(scp):
                ps = SPP.tile([128, 2, 512], F32, tag="sp", name="psv")
                for i in range(2):
                    sc = scp * 2 + i
                    for dc in range(DC):
                        nc.tensor.matmul(
                            ps[:, i, :],
                            lhsT=xT_sb[:, dc, sc * 128:(sc + 1) * 128],
                            rhs=w_sb["wv"][:, dc, :],
                            start=(dc == 0), stop=(dc == DC - 1))
                vdst = vp4[:, scp * 2:(scp + 1) * 2, :, 0:64]
                vsrc = ps.rearrange("p i (h c) -> p i h c", c=64)
                if scp % 2 == 0:
                    nc.vector.tensor_copy(vdst, vsrc)
                else:
                    nc.scalar.copy(out=vdst, in_=vsrc)

            # attnV emission lags the scores/exp stream by ATTNV_LAG groups
            # so the in-order PE queue never stalls waiting for praw/pf of
            # the group it just produced (software pipelining).
            ATTNV_LAG = 3

            def emit_attnv(cps_t, h, g, pf):
                for j in range(2):
                    kc = g * 2 + j
                    nc.tensor.matmul(
                        cps_t,
                        lhsT=vplus[:, kc, h * 65:(h + 1) * 65],
                        rhs=pf[:, j, :],
                        start=(kc == 0), stop=(kc == SC128 - 1))

            pending = None
            attnv_q = []
            for h in range(H):
                oc, rb = h // 2, (h % 2) * 64
                cps_h = CP.tile([65, QS], F32, tag="ctx", name="ctx")
                for g in range(8):
                    if h == 0 and g % 2 == 0:
                        emit_k_block(g // 2)
                    ps = SPP.tile([128, 2, 512], F32, tag="sp", name="pss")
                    for j in range(2):
                        kc = g * 2 + j
                        nc.tensor.matmul(
                            ps[:, j, :],
                            lhsT=kT_sb[rb:rb + 64, oc,
                                       kc * 128:(kc + 1) * 128],
                            rhs=qT_sb[rb:rb + 64, oc, :],
                            start=True, stop=True)
                    praw = PRW.tile([128, 2, 512], BF16, tag="praw",
                                    name="praw")
                    nc.scalar.activation(out=praw, in_=ps, func=AF.Exp,
                                         scale=0.125)
                    if h == 0:
                        cs = SPP.tile([128, 2, 512], F32, tag="sp",
                                      name="cs")
                        for j in range(2):
                            kc = g * 2 + j
                            nc.tensor.matmul(
                                cs[:, j, :],
                                lhsT=ohk[:, kc * 128:(kc + 1) * 128],
                                rhs=ohq,
                                start=True, stop=True)
                        combine_pair(g, cs)
                    pf = PW.tile([128, 2, 512], BF16, tag="pf", name="pf")
                    nc.vector.tensor_tensor(
                        out=pf, in0=praw,
                        in1=ebT[:, g * 2:(g + 1) * 2, :], op=AL.mult)
                    if h == 0:
                        emit_v_pair(g)
                    attnv_q.append((cps_h, h, g, pf))
                    if len(attnv_q) > ATTNV_LAG:
                        emit_attnv(*attnv_q.pop(0))
                    if pending is not None and 2 <= g <= 4:
                        norm_step(g - 2, *pending)
                        if g == 4:
                            pending = None
                pending = (h, cps_h)
            while attnv_q:
                emit_attnv(*attnv_q.pop(0))

            # cvec = Wo @ bv + bo  (rank-1 epilogue row)
            cps = SPP.tile([128, 2, 512], F32, tag="sp", name="cps")
            for dc in range(DC):
                nc.tensor.matmul(cps[0:1, 0, :], lhsT=bv_bf[:, dc:dc + 1],
                                 rhs=wo_sb[:, dc, :],
                                 start=(dc == 0), stop=(dc == DC - 1))
            nc.vector.tensor_tensor(out=cvec, in0=cps[0:1, 0, :], in1=bo_row,
                                    op=AL.add)

            # ---- output projection; dc 0..2 (heads 0..5) for the first
            # three row-tiles overlap h7's tail before its norm lands ----
            poms = []
            for m in range(3):
                pom = SPP.tile([128, 2, 512], F32, tag="sp", name="pom")
                for dc in range(3):
                    nc.tensor.matmul(
                        pom[:, 0, :],
                        lhsT=ctxT_sb[:, dc, m * 128:(m + 1) * 128],
                        rhs=wo_sb[:, dc, :],
                        start=(dc == 0), stop=False)
                poms.append(pom)

            for step in range(3):
                norm_step(step, *pending)

            def finish_pom(pom, m, dcs):
                for dc in dcs:
                    nc.tensor.matmul(
                        pom[:, 0, :],
                        lhsT=ctxT_sb[:, dc, m * 128:(m + 1) * 128],
                        rhs=wo_sb[:, dc, :],
                        start=(dc == 0), stop=False)
                nc.tensor.matmul(pom[:, 0, :], lhsT=ones_row, rhs=cvec,
                                 start=False, stop=True)
                nc.vector.tensor_copy(o_sb[:, m, :], pom[:, 0, :])

            o_sb = P.tile([128, QT, 512], BF16, tag="o_sb")
            for m in range(3):
                finish_pom(poms[m], m, [3])
            pom3 = SPP.tile([128, 2, 512], F32, tag="sp", name="pom")
            finish_pom(pom3, 3, [0, 1, 2, 3])
            nc.sync.dma_start(
                out=out.rearrange("(m p) s -> p m s", p=128), in_=o_sb)

    nc.compile()
    return nc


def _prep_in_maps(inputs, with_masks):
    x = np.asarray(inputs["x"], np.float32)
    ipa = np.asarray(inputs["ipa_affinity_bias"], np.float32)
    asc = np.asarray(inputs["assoc_bias"], np.float32)
    msk = np.asarray(inputs["attention_mask"], np.float32)
    cid = np.asarray(inputs["concept_ids"])
    kpm = np.asarray(inputs["key_padding_mask"])

    wT = {nm: np.ascontiguousarray(
        np.asarray(inputs[nm], np.float32).T.astype(BF))
        for nm in ("Wq", "Wk", "Wv", "Wo")}
    bias = {nm: np.asarray(inputs[nm], np.float32)
            for nm in ("bq", "bv", "bo")}
    kpos = np.arange(S, dtype=np.float32)
    misc = np.zeros((128, 24), np.float32)
    misc[:, 0:16] = kpos.reshape(SC128, 128).T
    misc[:, 16:20] = bias["bq"].reshape(DC, 128).T
    misc[:, 20:24] = bias["bv"].reshape(DC, 128).T

    xT = [np.ascontiguousarray(x[b].T.astype(BF)) for b in range(B)]
    F8NP = ml_dtypes.float8_e4m3
    ipaT = [[np.ascontiguousarray(
        ipa[b, q0:q0 + QS].T.astype(F8NP)) for q0 in range(0, S, QS)]
        for b in range(B)]
    ascT = [[np.ascontiguousarray(
        asc[b, q0:q0 + QS].T.astype(F8NP)) for q0 in range(0, S, QS)]
        for b in range(B)]
    cidq_f = np.where(cid >= 0, cid, -1).astype(BF)
    cidk_f = np.where(cid >= 0, cid, -2).astype(BF)
    kpm_add = np.where(kpm, np.float32(-1e30), np.float32(0.0))

    in_maps = []
    for c in range(N_CORES):
        b, qi = c // 4, c % 4
        q0 = qi * QS
        m = {
            "xT": xT[b],
            "xqT": np.ascontiguousarray(xT[b][:, q0:q0 + QS]),
            "wqT": wT["Wq"], "wkT": wT["Wk"],
            "wvT": wT["Wv"], "woT": wT["Wo"],
            "bo": bias["bo"], "misc": misc,
            "ipaT": ipaT[b][qi],
            "ascT": ascT[b][qi],
            "cidq": np.ascontiguousarray(cidq_f[b, q0:q0 + QS]),
            "cidk": np.ascontiguousarray(cidk_f[b]),
            "qpos": (q0 + np.arange(QS)).astype(np.float16),
        }
        if with_masks:
            m["mskT"] = np.ascontiguousarray(
                msk[q0:q0 + QS].T.astype(BF))
            m["kpmT"] = np.ascontiguousarray(
                kpm_add[b].reshape(SC128, 128).T)
        in_maps.append(m)
    return in_maps


def run(inputs, trace=False):
    msk = np.asarray(inputs["attention_mask"])
    kpm = np.asarray(inputs["key_padding_mask"])
    with_masks = bool(msk.any() or kpm.any())
    if with_masks not in _COMPILED:
        _COMPILED[with_masks] = _build(with_masks)
    nc = _COMPILED[with_masks]
    in_maps = _prep_in_maps(inputs, with_masks)
    kw = {}
    if trace:
        kw = dict(trace=True, trace_cores=list(range(N_CORES)))
    res = bass_utils.run_bass_kernel_spmd(
        nc, in_maps, core_ids=list(range(N_CORES)), **kw)
    out = np.empty((B, S, D), np.float32)
    for c in range(N_CORES):
        b, q0 = c // 4, (c % 4) * QS
        out[b, q0:q0 + QS] = np.asarray(res.results[c]["out"],
                                        dtype=np.float32)
    return out, res


def kernel(**inputs) -> np.ndarray:
    out, _ = run(inputs)
    return out


# revision 33
# speedup vs baseline: 1.0721x; 1.0721x over previous
"""Trainium2 Bass kernel for AssociativeIncrementalAttention.

Multi-head attention (B=2, S=2048, D=512, H=8, HD=64) with additive
[B,S,S] bias tensors, a concept-equality bias, and key-padding mask.

Sharding: 8 cores, fully data-parallel (no collectives).
  core c -> batch b = c//4, query rows q0 = (c%4)*512 .. q0+512.

v2 design notes (engine-work reduction vs v1):
  - All large DRAM inputs are host-cast to bf16 (halves HBM traffic and
    the startup DMA head; matmuls ran in bf16 anyway).
  - Concept bias (same concept id, both valid, off-diagonal) is computed
    on the PE as a one-hot inner product: cs = onehotK^T @ (0.5*onehotQ)
    over the 64 concept ids, 16 tiny matmuls; replaces ~35us of DVE
    compare work. Diagonal exclusion multiplies cs by (qpos != kpos) on
    the 4 key chunks that contain the diagonal (exact, incl. cid<0).
  - bk is dropped exactly: softmax over k is invariant to the q.bk and
    bq.bk terms of (q+bq).(k+bk); only bq.k survives, so bq stays on Q
    and K needs no bias.  bv/bo ride the output epilogue (cvec) since
    softmax rows sum to 1.
  - Scalar (Act) engine does exp only (plus tiny psq-identity); PSUM->
    SBUF copies are split between DVE/Act/GpSimd to balance busy time.
  - Softmax normalization deferred past attn@V (rowsum rides a ones
    column in V); applied as recip + partition_broadcast + multiply,
    interleaved into the next head's groups.
  - attention_mask / key_padding_mask are all-zero in the target inputs;
    the fast variant skips them (host-checked), a generic variant with
    both is compiled lazily if they are ever nonzero.

Self-contained: hardcodes shapes; host-side prep is layout-only
(slices/transposes) plus dtype casts and tiny metadata encodings
(concept ids -> bf16 sentinel values, position iotas).
"""

import sys

if "/opt/trn_rl_repo" not in sys.path:
    sys.path.insert(0, "/opt/trn_rl_repo")

import numpy as np
import ml_dtypes

import concourse.bass as bass
import concourse.tile as tile
from concourse import bacc, mybir
from concourse import bass_utils

B, S, D, H = 2, 2048, 512, 8
HD = D // H  # 64
N_CORES = 8
QS = 512          # query rows per core
QT = QS // 128    # 4 query tiles per core
DC = D // 128     # 4 contraction chunks
SC512 = S // 512  # 4
SC128 = S // 128  # 16
NC = 64           # number of concept ids
F32 = mybir.dt.float32
F16 = mybir.dt.float16
BF16 = mybir.dt.bfloat16

BF = ml_dtypes.bfloat16

_COMPILED = {}


def _build(with_masks: bool):
    nc = bacc.Bacc("TRN2", target_bir_lowering=False, debug=False,
                   num_devices=N_CORES)

    def din(name, shape, dt=BF16):
        return nc.dram_tensor(name, shape, dt, kind="ExternalInput").ap()

    xT = din("xT", [D, S])            # x[b].T
    xqT = din("xqT", [D, QS])         # x[b, q0:q0+QS].T
    wqT = din("wqT", [D, D])
    wkT = din("wkT", [D, D])
    wvT = din("wvT", [D, D])
    woT = din("woT", [D, D])
    bo = din("bo", [D], F32)
    misc = din("misc", [128, 24], F32)  # kposT | bq cols | bv cols
    F8 = mybir.dt.float8e4
    ipaT = din("ipaT", [S, QS], F8)   # biases ~N(0,0.1): e4m3 quant err
    ascT = din("ascT", [S, QS], F8)   # ~0.004 abs on scores, negligible
    if with_masks:
        mskT = din("mskT", [S, QS])
        kpmT = din("kpmT", [128, SC128], F32)
    cidq = din("cidq", [QS])          # concept ids (neg -> -1 sentinel)
    cidk = din("cidk", [S])           # concept ids (neg -> -2 sentinel)
    qpos = din("qpos", [QS], F16)
    out = nc.dram_tensor("out", [QS, D], BF16, kind="ExternalOutput").ap()

    AL = mybir.AluOpType
    AF = mybir.ActivationFunctionType

    def bcast_ap(src, parts):
        # partition-broadcast read: [[0,parts]] + original free dims
        return bass.AP(tensor=src.tensor, offset=src.offset,
                       ap=[[0, parts]] + list(src.ap))

    # which kc chunks contain diagonal cells depends on q0 (per-core), but
    # the kernel is compiled once for all cores. The diagonal columns are
    # found at runtime by comparing qpos/kpos tiles, so we simply apply
    # the (qpos != kpos) multiply on ALL chunks' pairs... that would cost
    # 16 extra ops; instead exploit that each core's diagonal lies in the
    # 4 chunks q0/128..q0/128+3 -- but q0 is per-core. Trick: qpos/kpos
    # comparison is data-driven, so apply the diag-multiply on all 8
    # pairs only if needed. Cheaper: host passes kpos shifted so that the
    # compare is false outside the diag chunks anyway; the multiply by
    # e in {0,1} is exact everywhere, so applying it everywhere is always
    # correct. We bound cost by applying per-pair (8 pairs) with a
    # [128,1]-scalar compare per chunk (16 ts + 16 tt).
    # -> final choice: apply on all chunks (uniform, data-driven, safe).

    with tile.TileContext(nc) as tc:
        with (
            tc.tile_pool(name="persist", bufs=1) as P,
            tc.tile_pool(name="ipain", bufs=2) as IPA,
            tc.tile_pool(name="ascin", bufs=2) as ASC,
            tc.tile_pool(name="xfall", bufs=2) as XF,
            tc.tile_pool(name="ework", bufs=3) as EW,
            tc.tile_pool(name="praww", bufs=8) as PRW,
            tc.tile_pool(name="pfw", bufs=8) as PW,
            tc.tile_pool(name="rswork", bufs=2) as RW,
            tc.tile_pool(name="osb", bufs=2) as OS,
            tc.tile_pool(name="spp", bufs=3, space="PSUM") as SPP,
            tc.tile_pool(name="ctxp", bufs=2, space="PSUM") as CP,
        ):
            # ---- persistent tiles ----
            kT_sb = P.tile([128, DC, S], BF16, tag="kT")
            qT_sb = P.tile([128, DC, QS], BF16, tag="qT")
            vplus = P.tile([128, SC128, 8 * 65], BF16, tag="vplus")
            vp4 = vplus.rearrange("p s (h c) -> p s h c", c=65)
            ebT = P.tile([128, SC128, QS], BF16, tag="ebT")
            ctxT_sb = P.tile([128, DC, QS], BF16, tag="ctxT")

            # ---- small input loads; wq/xqT lead the sync queue so the Q
            # projection unblocks first; scalar metadata rides one packed
            # [128,24] tensor (kpos | bq | bv) to cut per-DMA overheads ----
            w_sb = {}
            w_sb["wq"] = P.tile([128, DC, D], BF16, tag="wq", name="wq")
            nc.sync.dma_start(
                out=w_sb["wq"],
                in_=wqT.rearrange("(c p) s -> p c s", p=128))
            xqT_sb = P.tile([128, DC, QS], BF16, tag="xqT")
            nc.sync.dma_start(
                out=xqT_sb, in_=xqT.rearrange("(c p) s -> p c s", p=128))
            misc_sb = P.tile([128, 24], F32, tag="misc")
            nc.sync.dma_start(out=misc_sb, in_=misc)
            kposT_sb = misc_sb[:, 0:16]
            iota_col = misc_sb[0:NC, 0:1]
            cidkb = P.tile([NC, S], BF16, tag="cidkb")
            nc.sync.dma_start(out=cidkb, in_=bcast_ap(cidk, NC))
            cidqb = P.tile([NC, QS], BF16, tag="cidqb")
            nc.sync.dma_start(out=cidqb, in_=bcast_ap(cidq, NC))
            qposb = P.tile([128, QS], F16, tag="qposb")
            nc.sync.dma_start(out=qposb, in_=bcast_ap(qpos, 128))
            bo_row = P.tile([1, D], F32, tag="bo_row")
            nc.sync.dma_start(out=bo_row, in_=bo.rearrange("(a s) -> a s", a=1))
            if with_masks:
                kpmT_sb = P.tile([128, SC128], F32, tag="kpmT")
                nc.sync.dma_start(out=kpmT_sb, in_=kpmT)

            # ---- big input loads (gpsimd queue; issue order ~ arrival) ----
            def load_w(nm, ap_):
                w_sb[nm] = P.tile([128, DC, D], BF16, tag=nm, name=nm)
                nc.gpsimd.dma_start(
                    out=w_sb[nm], in_=ap_.rearrange("(c p) s -> p c s", p=128))

            load_w("wk", wkT)
            load_w("wv", wvT)
            xT_sb = P.tile([128, DC, S], BF16, tag="xT")

            def load_x_half(i):
                nc.gpsimd.dma_start(
                    out=xT_sb[:, :, i * 1024:(i + 1) * 1024],
                    in_=xT.rearrange("(c p) s -> p c s", p=128)[
                        :, :, i * 1024:(i + 1) * 1024])

            bias_in = {"ipa": [None] * 2, "asc": [None] * 2,
                       "msk": [None] * 2}

            def load_bias_g8(g8):
                specs = [("ipa", ipaT, IPA), ("asc", ascT, ASC)]
                if with_masks:
                    specs.append(("msk", mskT, EW))
                for nm, ap_, pool in specs:
                    t = pool.tile([128, 8, QS], BF16, tag="b" + nm, name=nm)
                    nc.gpsimd.dma_start(
                        out=t,
                        in_=ap_[g8 * 1024:(g8 + 1) * 1024, :].rearrange(
                            "(c p) s -> p c s", p=128))
                    bias_in[nm][g8] = t

            load_x_half(0)
            load_bias_g8(0)
            load_x_half(1)
            load_bias_g8(1)
            load_w("wo", woT)
            wo_sb = w_sb["wo"]

            bv_bf = P.tile([128, DC], BF16, tag="bv_bf")
            nc.vector.tensor_copy(bv_bf, misc_sb[:, 20:24])
            cvec = P.tile([1, D], BF16, tag="cvec")
            ones_row = P.tile([1, 128], BF16, tag="ones_row")
            nc.vector.memset(ones_row, 1.0)
            nc.vector.memset(vp4[:, :, :, 64:65], 1.0)

            # ---- one-hot concept encodings (DVE, tiny) ----
            ohk = P.tile([NC, S], BF16, tag="ohk")
            nc.vector.tensor_scalar(
                out=ohk, in0=cidkb, scalar1=iota_col, scalar2=None,
                op0=AL.is_equal)
            ohq = P.tile([NC, QS], BF16, tag="ohq")
            nc.vector.tensor_scalar(
                out=ohq, in0=cidqb, scalar1=iota_col, scalar2=0.5,
                op0=AL.is_equal, op1=AL.mult)

            # ---- concept-score matmuls; copied to SBUF right away so the
            # PSUM ring slots are not held hostage to the bias DMAs ----
            cs_sb = P.tile([128, SC128, QS], mybir.dt.float8e4,
                           tag="cs_sb")  # values in {0, 0.5}: exact in e4m3
            for p in range(8):
                cs = SPP.tile([128, 2, 512], F32, tag="sp", name="cs")
                for j in range(2):
                    kc = p * 2 + j
                    nc.tensor.matmul(
                        cs[:, j, :],
                        lhsT=ohk[:, kc * 128:(kc + 1) * 128],
                        rhs=ohq,
                        start=True, stop=True)
                dst = cs_sb[:, p * 2:(p + 1) * 2, :]
                if p % 2 == 0:
                    nc.vector.tensor_copy(dst, cs)
                else:
                    nc.scalar.copy(out=dst, in_=cs)

            # ---- combined bias -> exp, in [k, q] layout, per kc pair ----
            # xfall = cs*(qpos!=kpos) + ipa + asc (+ msk + kpm); eb = exp().
            def combine_pair(p):
                g8 = p // 4
                ipa_t = bias_in["ipa"][g8]
                asc_t = bias_in["asc"][g8]
                jj = (p % 4) * 2
                xf = XF.tile([128, 2, QS], BF16, tag="xf", name="xf")
                for j in range(2):
                    kc = p * 2 + j
                    # xf = (qpos != kpos) * cs  -- one fused op per chunk
                    nc.vector.scalar_tensor_tensor(
                        out=xf[:, j, :], in0=qposb,
                        scalar=kposT_sb[:, kc:kc + 1],
                        in1=cs_sb[:, p * 2 + j, :],
                        op0=AL.not_equal, op1=AL.mult)
                t1 = XF.tile([128, 2, QS], BF16, tag="t1", name="t1")
                nc.vector.tensor_tensor(
                    out=t1, in0=xf, in1=ipa_t[:, jj:jj + 2, :], op=AL.add)
                dst = XF.tile([128, 2, QS], BF16, tag="t2", name="t2")
                nc.vector.tensor_tensor(
                    out=dst, in0=t1, in1=asc_t[:, jj:jj + 2, :], op=AL.add)
                if with_masks:
                    dst2 = XF.tile([128, 2, QS], BF16, tag="t3", name="t3")
                    nc.vector.tensor_tensor(
                        out=dst2, in0=dst, in1=bias_in["msk"][g8][:, jj:jj + 2, :],
                        op=AL.add)
                    kcol = EW.tile([128, 2, QS], BF16, tag="kp", name="kp")
                    for j in range(2):
                        kc = p * 2 + j
                        nc.vector.tensor_scalar(
                            out=kcol[:, j, :], in0=dst2[:, j, :],
                            scalar1=kpmT_sb[:, kc:kc + 1], scalar2=None,
                            op0=AL.add)
                    dst = kcol
                # exp on Act
                nc.scalar.activation(
                    out=ebT[:, p * 2:(p + 1) * 2, :], in_=dst, func=AF.Exp)

            # ---- Q projection (+bq via Act identity) ----
            for ocp in range(2):
                ps = SPP.tile([128, 2, 512], F32, tag="sp", name="psq")
                for i in range(2):
                    oc = ocp * 2 + i
                    for dc in range(DC):
                        nc.tensor.matmul(
                            ps[:, i, :],
                            lhsT=w_sb["wq"][:, dc, oc * 128:(oc + 1) * 128],
                            rhs=xqT_sb[:, dc, :],
                            start=(dc == 0), stop=(dc == DC - 1))
                for i in range(2):
                    oc = ocp * 2 + i
                    nc.scalar.activation(
                        out=qT_sb[:, oc, :], in_=ps[:, i, :],
                        func=AF.Identity, bias=misc_sb[:, 16 + oc:17 + oc])

            # ---- attention main loop ----
            norm_state = {}

            def norm_step(step, hprev, cps_prev):
                ocp_, rbp = hprev // 2, (hprev % 2) * 64
                if step == 0:
                    rs_row = RW.tile([1, QS], F32, tag="rs_row",
                                     name="rs_row")
                    nc.vector.tensor_copy(rs_row, cps_prev[64:65, :])
                    rr = RW.tile([1, QS], F32, tag="rr", name="rr")
                    nc.vector.reciprocal_approx_fast(rr, rs_row)
                    norm_state["rr"] = rr
                elif step == 1:
                    rrb = RW.tile([64, QS], F32, tag="rrb", name="rrb")
                    nc.gpsimd.partition_broadcast(rrb, norm_state["rr"])
                    norm_state["rrb"] = rrb
                else:
                    nc.vector.tensor_tensor(
                        out=ctxT_sb[rbp:rbp + 64, ocp_, :],
                        in0=cps_prev[0:64, :], in1=norm_state["rrb"],
                        op=AL.mult)

            # ---- JIT emitters: K sc-block / V pair produced inside h0's
            # loop right before their first consumer, so no engine queue
            # entry waits on a DMA that lands later than its turn ----
            def emit_k_block(sc):
                for ocp in range(2):
                    ps = SPP.tile([128, 2, 512], F32, tag="sp", name="psk")
                    for i in range(2):
                        oc = ocp * 2 + i
                        for dc in range(DC):
                            nc.tensor.matmul(
                                ps[:, i, :],
                                lhsT=w_sb["wk"][:, dc,
                                                oc * 128:(oc + 1) * 128],
                                rhs=xT_sb[:, dc, sc * 512:(sc + 1) * 512],
                                start=(dc == 0), stop=(dc == DC - 1))
                    dst = kT_sb[:, ocp * 2:(ocp + 1) * 2,
                                sc * 512:(sc + 1) * 512]
                    if ocp == 0:
                        nc.vector.tensor_copy(dst, ps)
                    else:
                        nc.scalar.copy(out=dst, in_=ps)

            def emit_v_pair(scp):
                ps = SPP.tile([128, 2, 512], F32, tag="sp", name="psv")
                for i in range(2):
                    sc = scp * 2 + i
                    for dc in range(DC):
                        nc.tensor.matmul(
                            ps[:, i, :],
                            lhsT=xT_sb[:, dc, sc * 128:(sc + 1) * 128],
                            rhs=w_sb["wv"][:, dc, :],
                            start=(dc == 0), stop=(dc == DC - 1))
                vdst = vp4[:, scp * 2:(scp + 1) * 2, :, 0:64]
                vsrc = ps.rearrange("p i (h c) -> p i h c", c=64)
                if scp % 2 == 0:
                    nc.vector.tensor_copy(vdst, vsrc)
                else:
                    nc.scalar.copy(out=vdst, in_=vsrc)

            # attnV emission lags the scores/exp stream by ATTNV_LAG groups
            # so the in-order PE queue never stalls waiting for praw/pf of
            # the group it just produced (software pipelining).
            ATTNV_LAG = 3

            def emit_attnv(cps_t, h, g, pf):
                for j in range(2):
                    kc = g * 2 + j
                    nc.tensor.matmul(
                        cps_t,
                        lhsT=vplus[:, kc, h * 65:(h + 1) * 65],
                        rhs=pf[:, j, :],
                        start=(kc == 0), stop=(kc == SC128 - 1))

            pending = None
            attnv_q = []
            for h in range(H):
                oc, rb = h // 2, (h % 2) * 64
                cps_h = CP.tile([65, QS], F32, tag="ctx", name="ctx")
                for g in range(8):
                    if h == 0 and g % 2 == 0:
                        emit_k_block(g // 2)
                    ps = SPP.tile([128, 2, 512], F32, tag="sp", name="pss")
                    for j in range(2):
                        kc = g * 2 + j
                        nc.tensor.matmul(
                            ps[:, j, :],
                            lhsT=kT_sb[rb:rb + 64, oc,
                                       kc * 128:(kc + 1) * 128],
                            rhs=qT_sb[rb:rb + 64, oc, :],
                            start=True, stop=True)
                    praw = PRW.tile([128, 2, 512], BF16, tag="praw",
                                    name="praw")
                    nc.scalar.activation(out=praw, in_=ps, func=AF.Exp,
                                         scale=0.125)
                    if h == 0:
                        combine_pair(g)
                    pf = PW.tile([128, 2, 512], BF16, tag="pf", name="pf")
                    nc.vector.tensor_tensor(
                        out=pf, in0=praw,
                        in1=ebT[:, g * 2:(g + 1) * 2, :], op=AL.mult)
                    if h == 0:
                        emit_v_pair(g)
                    attnv_q.append((cps_h, h, g, pf))
                    if len(attnv_q) > ATTNV_LAG:
                        emit_attnv(*attnv_q.pop(0))
                    if pending is not None and 2 <= g <= 4:
                        norm_step(g - 2, *pending)
                        if g == 4:
                            pending = None
                pending = (h, cps_h)
            while attnv_q:
                emit_attnv(*attnv_q.pop(0))

            # cvec = Wo @ bv + bo  (rank-1 epilogue row)
            cps = SPP.tile([128, 2, 512], F32, tag="sp", name="cps")
            for dc in range(DC):
                nc.tensor.matmul(cps[0:1, 0, :], lhsT=bv_bf[:, dc:dc + 1],
                                 rhs=wo_sb[:, dc, :],
                                 start=(dc == 0), stop=(dc == DC - 1))
            nc.vector.tensor_tensor(out=cvec, in0=cps[0:1, 0, :], in1=bo_row,
                                    op=AL.add)

            # ---- output projection; dc 0..2 (heads 0..5) for the first
            # three row-tiles overlap h7's tail before its norm lands ----
            poms = []
            for m in range(3):
                pom = SPP.tile([128, 2, 512], F32, tag="sp", name="pom")
                for dc in range(3):
                    nc.tensor.matmul(
                        pom[:, 0, :],
                        lhsT=ctxT_sb[:, dc, m * 128:(m + 1) * 128],
                        rhs=wo_sb[:, dc, :],
                        start=(dc == 0), stop=False)
                poms.append(pom)

            for step in range(3):
                norm_step(step, *pending)

            def finish_pom(pom, m, dcs):
                for dc in dcs:
                    nc.tensor.matmul(
                        pom[:, 0, :],
                        lhsT=ctxT_sb[:, dc, m * 128:(m + 1) * 128],
                        rhs=wo_sb[:, dc, :],
                        start=(dc == 0), stop=False)
                nc.tensor.matmul(pom[:, 0, :], lhsT=ones_row, rhs=cvec,
                                 start=False, stop=True)
                nc.vector.tensor_copy(o_sb[:, m, :], pom[:, 0, :])

            o_sb = P.tile([128, QT, 512], BF16, tag="o_sb")
            for m in range(3):
                finish_pom(poms[m], m, [3])
            pom3 = SPP.tile([128, 2, 512], F32, tag="sp", name="pom")
            finish_pom(pom3, 3, [0, 1, 2, 3])
            nc.sync.dma_start(
                out=out.rearrange("(m p) s -> p m s", p=128), in_=o_sb)

    nc.compile()
    return nc


def _prep_in_maps(inputs, with_masks):
    x = np.asarray(inputs["x"], np.float32)
    ipa = np.asarray(inputs["ipa_affinity_bias"], np.float32)
    asc = np.asarray(inputs["assoc_bias"], np.float32)
    msk = np.asarray(inputs["attention_mask"], np.float32)
    cid = np.asarray(inputs["concept_ids"])
    kpm = np.asarray(inputs["key_padding_mask"])

    wT = {nm: np.ascontiguousarray(
        np.asarray(inputs[nm], np.float32).T.astype(BF))
        for nm in ("Wq", "Wk", "Wv", "Wo")}
    bias = {nm: np.asarray(inputs[nm], np.float32)
            for nm in ("bq", "bv", "bo")}
    kpos = np.arange(S, dtype=np.float32)
    misc = np.zeros((128, 24), np.float32)
    misc[:, 0:16] = kpos.reshape(SC128, 128).T
    misc[:, 16:20] = bias["bq"].reshape(DC, 128).T
    misc[:, 20:24] = bias["bv"].reshape(DC, 128).T

    xT = [np.ascontiguousarray(x[b].T.astype(BF)) for b in range(B)]
    F8NP = ml_dtypes.float8_e4m3
    ipaT = [[np.ascontiguousarray(
        ipa[b, q0:q0 + QS].T.astype(F8NP)) for q0 in range(0, S, QS)]
        for b in range(B)]
    ascT = [[np.ascontiguousarray(
        asc[b, q0:q0 + QS].T.astype(F8NP)) for q0 in range(0, S, QS)]
        for b in range(B)]
    cidq_f = np.where(cid >= 0, cid, -1).astype(BF)
    cidk_f = np.where(cid >= 0, cid, -2).astype(BF)
    kpm_add = np.where(kpm, np.float32(-1e30), np.float32(0.0))

    in_maps = []
    for c in range(N_CORES):
        b, qi = c // 4, c % 4
        q0 = qi * QS
        m = {
            "xT": xT[b],
            "xqT": np.ascontiguousarray(xT[b][:, q0:q0 + QS]),
            "wqT": wT["Wq"], "wkT": wT["Wk"],
            "wvT": wT["Wv"], "woT": wT["Wo"],
            "bo": bias["bo"], "misc": misc,
            "ipaT": ipaT[b][qi],
            "ascT": ascT[b][qi],
            "cidq": np.ascontiguousarray(cidq_f[b, q0:q0 + QS]),
            "cidk": np.ascontiguousarray(cidk_f[b]),
            "qpos": (q0 + np.arange(QS)).astype(np.float16),
        }
        if with_masks:
            m["mskT"] = np.ascontiguousarray(
                msk[q0:q0 + QS].T.astype(BF))
            m["kpmT"] = np.ascontiguousarray(
                kpm_add[b].reshape(SC128, 128).T)
        in_maps.append(m)
    return in_maps


def run(inputs, trace=False):
    msk = np.asarray(inputs["attention_mask"])
    kpm = np.asarray(inputs["key_padding_mask"])
    with_masks = bool(msk.any() or kpm.any())
    if with_masks not in _COMPILED:
        _COMPILED[with_masks] = _build(with_masks)
    nc = _COMPILED[with_masks]
    in_maps = _prep_in_maps(inputs, with_masks)
    kw = {}
    if trace:
        kw = dict(trace=True, trace_cores=list(range(N_CORES)))
    res = bass_utils.run_bass_kernel_spmd(
        nc, in_maps, core_ids=list(range(N_CORES)), **kw)
    out = np.empty((B, S, D), np.float32)
    for c in range(N_CORES):
        b, q0 = c // 4, (c % 4) * QS
        out[b, q0:q0 + QS] = np.asarray(res.results[c]["out"],
                                        dtype=np.float32)
    return out, res


def kernel(**inputs) -> np.ndarray:
    out, _ = run(inputs)
    return out


# revision 35
# speedup vs baseline: 1.2531x; 1.1689x over previous
"""Trainium2 Bass kernel for AssociativeIncrementalAttention.

Multi-head attention (B=2, S=2048, D=512, H=8, HD=64) with additive
[B,S,S] bias tensors, a concept-equality bias, and key-padding mask.

Sharding: 8 cores, fully data-parallel (no collectives).
  core c -> batch b = c//4, query rows q0 = (c%4)*512 .. q0+512.

v2 design notes (engine-work reduction vs v1):
  - All large DRAM inputs are host-cast to bf16 (halves HBM traffic and
    the startup DMA head; matmuls ran in bf16 anyway).
  - Concept bias (same concept id, both valid, off-diagonal) is computed
    on the PE as a one-hot inner product: cs = onehotK^T @ (0.5*onehotQ)
    over the 64 concept ids, 16 tiny matmuls; replaces ~35us of DVE
    compare work. Diagonal exclusion multiplies cs by (qpos != kpos) on
    the 4 key chunks that contain the diagonal (exact, incl. cid<0).
  - bk is dropped exactly: softmax over k is invariant to the q.bk and
    bq.bk terms of (q+bq).(k+bk); only bq.k survives, so bq stays on Q
    and K needs no bias.  bv/bo ride the output epilogue (cvec) since
    softmax rows sum to 1.
  - Scalar (Act) engine does exp only (plus tiny psq-identity); PSUM->
    SBUF copies are split between DVE/Act/GpSimd to balance busy time.
  - Softmax normalization deferred past attn@V (rowsum rides a ones
    column in V); applied as recip + partition_broadcast + multiply,
    interleaved into the next head's groups.
  - attention_mask / key_padding_mask are all-zero in the target inputs;
    the fast variant skips them (host-checked), a generic variant with
    both is compiled lazily if they are ever nonzero.

Self-contained: hardcodes shapes; host-side prep is layout-only
(slices/transposes) plus dtype casts and tiny metadata encodings
(concept ids -> bf16 sentinel values, position iotas).
"""

import sys

if "/opt/trn_rl_repo" not in sys.path:
    sys.path.insert(0, "/opt/trn_rl_repo")

import numpy as np
import ml_dtypes

import concourse.bass as bass
import concourse.tile as tile
from concourse import bacc, mybir
from concourse import bass_utils

B, S, D, H = 2, 2048, 512, 8
HD = D // H  # 64
N_CORES = 8
QS = 512          # query rows per core
QT = QS // 128    # 4 query tiles per core
DC = D // 128     # 4 contraction chunks
SC512 = S // 512  # 4
SC128 = S // 128  # 16
NC = 64           # number of concept ids
F32 = mybir.dt.float32
F16 = mybir.dt.float16
BF16 = mybir.dt.bfloat16

BF = ml_dtypes.bfloat16

_COMPILED = {}


def _build(with_masks: bool):
    nc = bacc.Bacc("TRN2", target_bir_lowering=False, debug=False,
                   num_devices=N_CORES)

    def din(name, shape, dt=BF16):
        return nc.dram_tensor(name, shape, dt, kind="ExternalInput").ap()

    xT = din("xT", [D, S])            # x[b].T
    xqT = din("xqT", [D, QS])         # x[b, q0:q0+QS].T
    wqT = din("wqT", [D, D])
    wkT = din("wkT", [D, D])
    wvT = din("wvT", [D, D])
    woT = din("woT", [D, D])
    bo = din("bo", [D], F32)
    misc = din("misc", [128, 24], F32)  # kposT | bq cols | bv cols
    F8 = mybir.dt.float8e4
    # ipa/asc interleaved [S, 2, QS]: one transfer per half instead of
    # two (per-DMA overhead ~1.2us dominates at these sizes); e4m3 quant
    # err ~0.004 abs on scores, negligible
    biasT = din("biasT", [S, 2, QS], F8)
    if with_masks:
        mskT = din("mskT", [S, QS])
        kpmT = din("kpmT", [128, SC128], F32)
    # cidk (neg -> -2) | cidq (neg -> -1) | qpos, all exact in f16
    meta = din("meta", [S + 2 * QS], F16)
    out = nc.dram_tensor("out", [QS, D], BF16, kind="ExternalOutput").ap()

    AL = mybir.AluOpType
    AF = mybir.ActivationFunctionType

    def bcast_ap(src, parts):
        # partition-broadcast read: [[0,parts]] + original free dims
        return bass.AP(tensor=src.tensor, offset=src.offset,
                       ap=[[0, parts]] + list(src.ap))

    # which kc chunks contain diagonal cells depends on q0 (per-core), but
    # the kernel is compiled once for all cores. The diagonal columns are
    # found at runtime by comparing qpos/kpos tiles, so we simply apply
    # the (qpos != kpos) multiply on ALL chunks' pairs... that would cost
    # 16 extra ops; instead exploit that each core's diagonal lies in the
    # 4 chunks q0/128..q0/128+3 -- but q0 is per-core. Trick: qpos/kpos
    # comparison is data-driven, so apply the diag-multiply on all 8
    # pairs only if needed. Cheaper: host passes kpos shifted so that the
    # compare is false outside the diag chunks anyway; the multiply by
    # e in {0,1} is exact everywhere, so applying it everywhere is always
    # correct. We bound cost by applying per-pair (8 pairs) with a
    # [128,1]-scalar compare per chunk (16 ts + 16 tt).
    # -> final choice: apply on all chunks (uniform, data-driven, safe).

    with tile.TileContext(nc) as tc:
        with (
            tc.tile_pool(name="persist", bufs=1) as P,
            tc.tile_pool(name="ipain", bufs=2) as IPA,
            tc.tile_pool(name="xfall", bufs=2) as XF,
            tc.tile_pool(name="ework", bufs=3) as EW,
            tc.tile_pool(name="praww", bufs=8) as PRW,
            tc.tile_pool(name="pfw", bufs=8) as PW,
            tc.tile_pool(name="rswork", bufs=2) as RW,
            tc.tile_pool(name="osb", bufs=2) as OS,
            tc.tile_pool(name="spp", bufs=3, space="PSUM") as SPP,
            tc.tile_pool(name="ctxp", bufs=2, space="PSUM") as CP,
        ):
            # ---- persistent tiles ----
            kT_sb = P.tile([128, DC, S], BF16, tag="kT")
            qT_sb = P.tile([128, DC, QS], BF16, tag="qT")
            vplus = P.tile([128, SC128, 8 * 65], BF16, tag="vplus")
            vp4 = vplus.rearrange("p s (h c) -> p s h c", c=65)
            ebT = P.tile([128, SC128, QS], BF16, tag="ebT")
            ctxT_sb = P.tile([128, DC, QS], BF16, tag="ctxT")

            # ---- small input loads; wq/xqT lead the sync queue so the Q
            # projection unblocks first; scalar metadata rides one packed
            # [128,24] tensor (kpos | bq | bv) to cut per-DMA overheads ----
            w_sb = {}
            w_sb["wq"] = P.tile([128, DC, D], BF16, tag="wq", name="wq")
            nc.sync.dma_start(
                out=w_sb["wq"],
                in_=wqT.rearrange("(c p) s -> p c s", p=128))
            xqT_sb = P.tile([128, DC, QS], BF16, tag="xqT")
            nc.sync.dma_start(
                out=xqT_sb, in_=xqT.rearrange("(c p) s -> p c s", p=128))
            misc_sb = P.tile([128, 24], F32, tag="misc")
            nc.sync.dma_start(out=misc_sb, in_=misc)
            kposT_sb = misc_sb[:, 0:16]
            iota_col = misc_sb[0:NC, 0:1]
            metab = P.tile([128, S + 2 * QS], F16, tag="metab")
            nc.sync.dma_start(out=metab, in_=bcast_ap(meta, 128))
            cidkb = metab[0:NC, 0:S]
            cidqb = metab[0:NC, S:S + QS]
            qposb = metab[:, S + QS:S + 2 * QS]
            bo_row = P.tile([1, D], F32, tag="bo_row")
            nc.sync.dma_start(out=bo_row, in_=bo.rearrange("(a s) -> a s", a=1))
            if with_masks:
                kpmT_sb = P.tile([128, SC128], F32, tag="kpmT")
                nc.sync.dma_start(out=kpmT_sb, in_=kpmT)

            # ---- big input loads (gpsimd queue; issue order ~ arrival) ----
            def load_w(nm, ap_):
                w_sb[nm] = P.tile([128, DC, D], BF16, tag=nm, name=nm)
                nc.gpsimd.dma_start(
                    out=w_sb[nm], in_=ap_.rearrange("(c p) s -> p c s", p=128))

            load_w("wk", wkT)
            load_w("wv", wvT)
            xT_sb = P.tile([128, DC, S], BF16, tag="xT")

            def load_x_half(i):
                nc.gpsimd.dma_start(
                    out=xT_sb[:, :, i * 1024:(i + 1) * 1024],
                    in_=xT.rearrange("(c p) s -> p c s", p=128)[
                        :, :, i * 1024:(i + 1) * 1024])

            bias_in = {"b": [None] * 2, "msk": [None] * 2}

            def load_bias_g8(g8):
                t = IPA.tile([128, 8, 2, QS], BF16, tag="bb", name="bb")
                nc.gpsimd.dma_start(
                    out=t,
                    in_=biasT[g8 * 1024:(g8 + 1) * 1024].rearrange(
                        "(c p) t s -> p c t s", p=128))
                bias_in["b"][g8] = t
                if with_masks:
                    tm = EW.tile([128, 8, QS], BF16, tag="bmsk", name="msk")
                    nc.gpsimd.dma_start(
                        out=tm,
                        in_=mskT[g8 * 1024:(g8 + 1) * 1024, :].rearrange(
                            "(c p) s -> p c s", p=128))
                    bias_in["msk"][g8] = tm

            load_x_half(0)
            load_bias_g8(0)
            load_x_half(1)
            load_bias_g8(1)
            load_w("wo", woT)
            wo_sb = w_sb["wo"]

            bv_bf = P.tile([128, DC], BF16, tag="bv_bf")
            nc.vector.tensor_copy(bv_bf, misc_sb[:, 20:24])
            cvec = P.tile([1, D], BF16, tag="cvec")
            ones_row = P.tile([1, 128], BF16, tag="ones_row")
            nc.vector.memset(ones_row, 1.0)
            nc.vector.memset(vp4[:, :, :, 64:65], 1.0)

            # ---- one-hot concept encodings (DVE, tiny) ----
            ohk = P.tile([NC, S], BF16, tag="ohk")
            nc.vector.tensor_scalar(
                out=ohk, in0=cidkb, scalar1=iota_col, scalar2=None,
                op0=AL.is_equal)
            ohq = P.tile([NC, QS], BF16, tag="ohq")
            nc.vector.tensor_scalar(
                out=ohq, in0=cidqb, scalar1=iota_col, scalar2=0.5,
                op0=AL.is_equal, op1=AL.mult)

            # ---- concept-score matmuls; copied to SBUF right away so the
            # PSUM ring slots are not held hostage to the bias DMAs ----
            cs_sb = P.tile([128, SC128, QS], mybir.dt.float8e4,
                           tag="cs_sb")  # values in {0, 0.5}: exact in e4m3
            for p in range(8):
                cs = SPP.tile([128, 2, 512], F32, tag="sp", name="cs")
                for j in range(2):
                    kc = p * 2 + j
                    nc.tensor.matmul(
                        cs[:, j, :],
                        lhsT=ohk[:, kc * 128:(kc + 1) * 128],
                        rhs=ohq,
                        start=True, stop=True)
                dst = cs_sb[:, p * 2:(p + 1) * 2, :]
                if p % 2 == 0:
                    nc.vector.tensor_copy(dst, cs)
                else:
                    nc.scalar.copy(out=dst, in_=cs)

            # ---- combined bias -> exp, in [k, q] layout, per kc pair ----
            # xfall = cs*(qpos!=kpos) + ipa + asc (+ msk + kpm); eb = exp().
            def combine_pair(p):
                g8 = p // 4
                bt = bias_in["b"][g8]
                jj = (p % 4) * 2
                xf = XF.tile([128, 2, QS], BF16, tag="xf", name="xf")
                for j in range(2):
                    kc = p * 2 + j
                    # xf = (qpos != kpos) * cs  -- one fused op per chunk
                    nc.vector.scalar_tensor_tensor(
                        out=xf[:, j, :], in0=qposb,
                        scalar=kposT_sb[:, kc:kc + 1],
                        in1=cs_sb[:, p * 2 + j, :],
                        op0=AL.not_equal, op1=AL.mult)
                t1 = XF.tile([128, 2, QS], BF16, tag="t1", name="t1")
                nc.vector.tensor_tensor(
                    out=t1, in0=xf, in1=bt[:, jj:jj + 2, 0, :], op=AL.add)
                dst = XF.tile([128, 2, QS], BF16, tag="t2", name="t2")
                nc.vector.tensor_tensor(
                    out=dst, in0=t1, in1=bt[:, jj:jj + 2, 1, :], op=AL.add)
                if with_masks:
                    dst2 = XF.tile([128, 2, QS], BF16, tag="t3", name="t3")
                    nc.vector.tensor_tensor(
                        out=dst2, in0=dst, in1=bias_in["msk"][g8][:, jj:jj + 2, :],
                        op=AL.add)
                    kcol = EW.tile([128, 2, QS], BF16, tag="kp", name="kp")
                    for j in range(2):
                        kc = p * 2 + j
                        nc.vector.tensor_scalar(
                            out=kcol[:, j, :], in0=dst2[:, j, :],
                            scalar1=kpmT_sb[:, kc:kc + 1], scalar2=None,
                            op0=AL.add)
                    dst = kcol
                # exp on Act
                nc.scalar.activation(
                    out=ebT[:, p * 2:(p + 1) * 2, :], in_=dst, func=AF.Exp)

            # ---- Q projection (+bq via Act identity) ----
            for ocp in range(2):
                ps = SPP.tile([128, 2, 512], F32, tag="sp", name="psq")
                for i in range(2):
                    oc = ocp * 2 + i
                    for dc in range(DC):
                        nc.tensor.matmul(
                            ps[:, i, :],
                            lhsT=w_sb["wq"][:, dc, oc * 128:(oc + 1) * 128],
                            rhs=xqT_sb[:, dc, :],
                            start=(dc == 0), stop=(dc == DC - 1))
                for i in range(2):
                    oc = ocp * 2 + i
                    nc.scalar.activation(
                        out=qT_sb[:, oc, :], in_=ps[:, i, :],
                        func=AF.Identity, bias=misc_sb[:, 16 + oc:17 + oc])

            # ---- attention main loop ----
            norm_state = {}

            def norm_step(step, hprev, cps_prev):
                ocp_, rbp = hprev // 2, (hprev % 2) * 64
                if step == 0:
                    rs_row = RW.tile([1, QS], F32, tag="rs_row",
                                     name="rs_row")
                    nc.vector.tensor_copy(rs_row, cps_prev[64:65, :])
                    rr = RW.tile([1, QS], F32, tag="rr", name="rr")
                    nc.vector.reciprocal_approx_fast(rr, rs_row)
                    norm_state["rr"] = rr
                elif step == 1:
                    rrb = RW.tile([64, QS], F32, tag="rrb", name="rrb")
                    nc.gpsimd.partition_broadcast(rrb, norm_state["rr"])
                    norm_state["rrb"] = rrb
                else:
                    nc.vector.tensor_tensor(
                        out=ctxT_sb[rbp:rbp + 64, ocp_, :],
                        in0=cps_prev[0:64, :], in1=norm_state["rrb"],
                        op=AL.mult)

            # ---- JIT emitters: K sc-block / V pair produced inside h0's
            # loop right before their first consumer, so no engine queue
            # entry waits on a DMA that lands later than its turn ----
            def emit_k_block(sc):
                for ocp in range(2):
                    ps = SPP.tile([128, 2, 512], F32, tag="sp", name="psk")
                    for i in range(2):
                        oc = ocp * 2 + i
                        for dc in range(DC):
                            nc.tensor.matmul(
                                ps[:, i, :],
                                lhsT=w_sb["wk"][:, dc,
                                                oc * 128:(oc + 1) * 128],
                                rhs=xT_sb[:, dc, sc * 512:(sc + 1) * 512],
                                start=(dc == 0), stop=(dc == DC - 1))
                    dst = kT_sb[:, ocp * 2:(ocp + 1) * 2,
                                sc * 512:(sc + 1) * 512]
                    if ocp == 0:
                        nc.vector.tensor_copy(dst, ps)
                    else:
                        nc.scalar.copy(out=dst, in_=ps)

            def emit_v_pair(scp):
                ps = SPP.tile([128, 2, 512], F32, tag="sp", name="psv")
                for i in range(2):
                    sc = scp * 2 + i
                    for dc in range(DC):
                        nc.tensor.matmul(
                            ps[:, i, :],
                            lhsT=xT_sb[:, dc, sc * 128:(sc + 1) * 128],
                            rhs=w_sb["wv"][:, dc, :],
                            start=(dc == 0), stop=(dc == DC - 1))
                vdst = vp4[:, scp * 2:(scp + 1) * 2, :, 0:64]
                vsrc = ps.rearrange("p i (h c) -> p i h c", c=64)
                if scp % 2 == 0:
                    nc.vector.tensor_copy(vdst, vsrc)
                else:
                    nc.scalar.copy(out=vdst, in_=vsrc)

            # attnV emission lags the scores/exp stream by ATTNV_LAG groups
            # so the in-order PE queue never stalls waiting for praw/pf of
            # the group it just produced (software pipelining).
            ATTNV_LAG = 3

            def emit_attnv(cps_t, h, g, pf):
                for j in range(2):
                    kc = g * 2 + j
                    nc.tensor.matmul(
                        cps_t,
                        lhsT=vplus[:, kc, h * 65:(h + 1) * 65],
                        rhs=pf[:, j, :],
                        start=(kc == 0), stop=(kc == SC128 - 1))

            pending = None
            attnv_q = []
            for h in range(H):
                oc, rb = h // 2, (h % 2) * 64
                cps_h = CP.tile([65, QS], F32, tag="ctx", name="ctx")
                for g in range(8):
                    if h == 0 and g % 2 == 0:
                        emit_k_block(g // 2)
                    ps = SPP.tile([128, 2, 512], F32, tag="sp", name="pss")
                    for j in range(2):
                        kc = g * 2 + j
                        nc.tensor.matmul(
                            ps[:, j, :],
                            lhsT=kT_sb[rb:rb + 64, oc,
                                       kc * 128:(kc + 1) * 128],
                            rhs=qT_sb[rb:rb + 64, oc, :],
                            start=True, stop=True)
                    praw = PRW.tile([128, 2, 512], BF16, tag="praw",
                                    name="praw")
                    nc.scalar.activation(out=praw, in_=ps, func=AF.Exp,
                                         scale=0.125)
                    if h == 0:
                        combine_pair(g)
                    pf = PW.tile([128, 2, 512], BF16, tag="pf", name="pf")
                    nc.vector.tensor_tensor(
                        out=pf, in0=praw,
                        in1=ebT[:, g * 2:(g + 1) * 2, :], op=AL.mult)
                    if h == 0:
                        emit_v_pair(g)
                    attnv_q.append((cps_h, h, g, pf))
                    if len(attnv_q) > ATTNV_LAG:
                        emit_attnv(*attnv_q.pop(0))
                    if pending is not None and 2 <= g <= 4:
                        norm_step(g - 2, *pending)
                        if g == 4:
                            pending = None
                pending = (h, cps_h)
            while attnv_q:
                emit_attnv(*attnv_q.pop(0))

            # cvec = Wo @ bv + bo  (rank-1 epilogue row)
            cps = SPP.tile([128, 2, 512], F32, tag="sp", name="cps")
            for dc in range(DC):
                nc.tensor.matmul(cps[0:1, 0, :], lhsT=bv_bf[:, dc:dc + 1],
                                 rhs=wo_sb[:, dc, :],
                                 start=(dc == 0), stop=(dc == DC - 1))
            nc.vector.tensor_tensor(out=cvec, in0=cps[0:1, 0, :], in1=bo_row,
                                    op=AL.add)

            # ---- output projection; dc 0..2 (heads 0..5) for the first
            # three row-tiles overlap h7's tail before its norm lands ----
            poms = []
            for m in range(3):
                pom = SPP.tile([128, 2, 512], F32, tag="sp", name="pom")
                for dc in range(3):
                    nc.tensor.matmul(
                        pom[:, 0, :],
                        lhsT=ctxT_sb[:, dc, m * 128:(m + 1) * 128],
                        rhs=wo_sb[:, dc, :],
                        start=(dc == 0), stop=False)
                poms.append(pom)

            for step in range(3):
                norm_step(step, *pending)

            def finish_pom(pom, m, dcs):
                for dc in dcs:
                    nc.tensor.matmul(
                        pom[:, 0, :],
                        lhsT=ctxT_sb[:, dc, m * 128:(m + 1) * 128],
                        rhs=wo_sb[:, dc, :],
                        start=(dc == 0), stop=False)
                nc.tensor.matmul(pom[:, 0, :], lhsT=ones_row, rhs=cvec,
                                 start=False, stop=True)
                nc.vector.tensor_copy(o_sb[:, m, :], pom[:, 0, :])

            o_sb = P.tile([128, QT, 512], BF16, tag="o_sb")
            for m in range(3):
                finish_pom(poms[m], m, [3])
            pom3 = SPP.tile([128, 2, 512], F32, tag="sp", name="pom")
            finish_pom(pom3, 3, [0, 1, 2, 3])
            nc.sync.dma_start(
                out=out.rearrange("(m p) s -> p m s", p=128), in_=o_sb)

    nc.compile()
    return nc


def _prep_in_maps(inputs, with_masks):
    x = np.asarray(inputs["x"], np.float32)
    ipa = np.asarray(inputs["ipa_affinity_bias"], np.float32)
    asc = np.asarray(inputs["assoc_bias"], np.float32)
    msk = np.asarray(inputs["attention_mask"], np.float32)
    cid = np.asarray(inputs["concept_ids"])
    kpm = np.asarray(inputs["key_padding_mask"])

    wT = {nm: np.ascontiguousarray(
        np.asarray(inputs[nm], np.float32).T.astype(BF))
        for nm in ("Wq", "Wk", "Wv", "Wo")}
    bias = {nm: np.asarray(inputs[nm], np.float32)
            for nm in ("bq", "bv", "bo")}
    kpos = np.arange(S, dtype=np.float32)
    misc = np.zeros((128, 24), np.float32)
    misc[:, 0:16] = kpos.reshape(SC128, 128).T
    misc[:, 16:20] = bias["bq"].reshape(DC, 128).T
    misc[:, 20:24] = bias["bv"].reshape(DC, 128).T

    xT = [np.ascontiguousarray(x[b].T.astype(BF)) for b in range(B)]
    F8NP = ml_dtypes.float8_e4m3
    biasT = [[np.ascontiguousarray(np.stack(
        [ipa[b, q0:q0 + QS].T, asc[b, q0:q0 + QS].T],
        axis=1).astype(F8NP)) for q0 in range(0, S, QS)]
        for b in range(B)]
    cidq_f = np.where(cid >= 0, cid, -1).astype(np.float32)
    cidk_f = np.where(cid >= 0, cid, -2).astype(np.float32)
    kpm_add = np.where(kpm, np.float32(-1e30), np.float32(0.0))

    in_maps = []
    for c in range(N_CORES):
        b, qi = c // 4, c % 4
        q0 = qi * QS
        m = {
            "xT": xT[b],
            "xqT": np.ascontiguousarray(xT[b][:, q0:q0 + QS]),
            "wqT": wT["Wq"], "wkT": wT["Wk"],
            "wvT": wT["Wv"], "woT": wT["Wo"],
            "bo": bias["bo"], "misc": misc,
            "biasT": biasT[b][qi],
            "meta": np.concatenate([
                cidk_f[b], cidq_f[b, q0:q0 + QS],
                (q0 + np.arange(QS)).astype(np.float32)]).astype(
                    np.float16),
        }
        if with_masks:
            m["mskT"] = np.ascontiguousarray(
                msk[q0:q0 + QS].T.astype(BF))
            m["kpmT"] = np.ascontiguousarray(
                kpm_add[b].reshape(SC128, 128).T)
        in_maps.append(m)
    return in_maps


def run(inputs, trace=False):
    msk = np.asarray(inputs["attention_mask"])
    kpm = np.asarray(inputs["key_padding_mask"])
    with_masks = bool(msk.any() or kpm.any())
    if with_masks not in _COMPILED:
        _COMPILED[with_masks] = _build(with_masks)
    nc = _COMPILED[with_masks]
    in_maps = _prep_in_maps(inputs, with_masks)
    kw = {}
    if trace:
        kw = dict(trace=True, trace_cores=list(range(N_CORES)))
    res = bass_utils.run_bass_kernel_spmd(
        nc, in_maps, core_ids=list(range(N_CORES)), **kw)
    out = np.empty((B, S, D), np.float32)
    for c in range(N_CORES):
        b, q0 = c // 4, (c % 4) * QS
        out[b, q0:q0 + QS] = np.asarray(res.results[c]["out"],
                                        dtype=np.float32)
    return out, res


def kernel(**inputs) -> np.ndarray:
    out, _ = run(inputs)
    return out


# revision 37
# speedup vs baseline: 1.3127x; 1.0476x over previous
"""Trainium2 Bass kernel for AssociativeIncrementalAttention.

Multi-head attention (B=2, S=2048, D=512, H=8, HD=64) with additive
[B,S,S] bias tensors, a concept-equality bias, and key-padding mask.

Sharding: 8 cores, fully data-parallel (no collectives).
  core c -> batch b = c//4, query rows q0 = (c%4)*512 .. q0+512.

v2 design notes (engine-work reduction vs v1):
  - All large DRAM inputs are host-cast to bf16 (halves HBM traffic and
    the startup DMA head; matmuls ran in bf16 anyway).
  - Concept bias (same concept id, both valid, off-diagonal) is computed
    on the PE as a one-hot inner product: cs = onehotK^T @ (0.5*onehotQ)
    over the 64 concept ids, 16 tiny matmuls; replaces ~35us of DVE
    compare work. Diagonal exclusion multiplies cs by (qpos != kpos) on
    the 4 key chunks that contain the diagonal (exact, incl. cid<0).
  - bk is dropped exactly: softmax over k is invariant to the q.bk and
    bq.bk terms of (q+bq).(k+bk); only bq.k survives, so bq stays on Q
    and K needs no bias.  bv/bo ride the output epilogue (cvec) since
    softmax rows sum to 1.
  - Scalar (Act) engine does exp only (plus tiny psq-identity); PSUM->
    SBUF copies are split between DVE/Act/GpSimd to balance busy time.
  - Softmax normalization deferred past attn@V (rowsum rides a ones
    column in V); applied as recip + partition_broadcast + multiply,
    interleaved into the next head's groups.
  - attention_mask / key_padding_mask are all-zero in the target inputs;
    the fast variant skips them (host-checked), a generic variant with
    both is compiled lazily if they are ever nonzero.

Self-contained: hardcodes shapes; host-side prep is layout-only
(slices/transposes) plus dtype casts and tiny metadata encodings
(concept ids -> bf16 sentinel values, position iotas).
"""

import sys

if "/opt/trn_rl_repo" not in sys.path:
    sys.path.insert(0, "/opt/trn_rl_repo")

import numpy as np
import ml_dtypes

import concourse.bass as bass
import concourse.tile as tile
from concourse import bacc, mybir
from concourse import bass_utils

B, S, D, H = 2, 2048, 512, 8
HD = D // H  # 64
N_CORES = 8
QS = 512          # query rows per core
QT = QS // 128    # 4 query tiles per core
DC = D // 128     # 4 contraction chunks
SC512 = S // 512  # 4
SC128 = S // 128  # 16
NC = 64           # number of concept ids
F32 = mybir.dt.float32
F16 = mybir.dt.float16
BF16 = mybir.dt.bfloat16

BF = ml_dtypes.bfloat16

_COMPILED = {}


def _build(with_masks: bool):
    nc = bacc.Bacc("TRN2", target_bir_lowering=False, debug=False,
                   num_devices=N_CORES)

    def din(name, shape, dt=BF16):
        return nc.dram_tensor(name, shape, dt, kind="ExternalInput").ap()

    xT = din("xT", [D, S])            # x[b].T
    xqT = din("xqT", [D, QS])         # x[b, q0:q0+QS].T
    wqT = din("wqT", [D, D])
    wkT = din("wkT", [D, D])
    wvT = din("wvT", [D, D])
    woT = din("woT", [D, D])
    bo = din("bo", [D], F32)
    misc = din("misc", [128, 24], F32)  # kposT | bq cols | bv cols
    F8 = mybir.dt.float8e4
    ipaT = din("ipaT", [S, QS], F8)   # biases ~N(0,0.1): e4m3 quant err
    ascT = din("ascT", [S, QS], F8)   # ~0.004 abs on scores, negligible
    if with_masks:
        mskT = din("mskT", [S, QS])
        kpmT = din("kpmT", [128, SC128], F32)
    cidq = din("cidq", [QS])          # concept ids (neg -> -1 sentinel)
    cidk = din("cidk", [S])           # concept ids (neg -> -2 sentinel)
    qpos = din("qpos", [QS], F16)
    out = nc.dram_tensor("out", [QS, D], BF16, kind="ExternalOutput").ap()

    AL = mybir.AluOpType
    AF = mybir.ActivationFunctionType

    def bcast_ap(src, parts):
        # partition-broadcast read: [[0,parts]] + original free dims
        return bass.AP(tensor=src.tensor, offset=src.offset,
                       ap=[[0, parts]] + list(src.ap))

    # which kc chunks contain diagonal cells depends on q0 (per-core), but
    # the kernel is compiled once for all cores. The diagonal columns are
    # found at runtime by comparing qpos/kpos tiles, so we simply apply
    # the (qpos != kpos) multiply on ALL chunks' pairs... that would cost
    # 16 extra ops; instead exploit that each core's diagonal lies in the
    # 4 chunks q0/128..q0/128+3 -- but q0 is per-core. Trick: qpos/kpos
    # comparison is data-driven, so apply the diag-multiply on all 8
    # pairs only if needed. Cheaper: host passes kpos shifted so that the
    # compare is false outside the diag chunks anyway; the multiply by
    # e in {0,1} is exact everywhere, so applying it everywhere is always
    # correct. We bound cost by applying per-pair (8 pairs) with a
    # [128,1]-scalar compare per chunk (16 ts + 16 tt).
    # -> final choice: apply on all chunks (uniform, data-driven, safe).

    with tile.TileContext(nc) as tc:
        with (
            tc.tile_pool(name="persist", bufs=1) as P,
            tc.tile_pool(name="ipain", bufs=2) as IPA,
            tc.tile_pool(name="ascin", bufs=2) as ASC,
            tc.tile_pool(name="xfall", bufs=2) as XF,
            tc.tile_pool(name="ework", bufs=3) as EW,
            tc.tile_pool(name="praww", bufs=8) as PRW,
            tc.tile_pool(name="pfw", bufs=8) as PW,
            tc.tile_pool(name="rswork", bufs=2) as RW,
            tc.tile_pool(name="osb", bufs=2) as OS,
            tc.tile_pool(name="spp", bufs=3, space="PSUM") as SPP,
            tc.tile_pool(name="ctxp", bufs=2, space="PSUM") as CP,
        ):
            # ---- persistent tiles ----
            kT_sb = P.tile([128, DC, S], BF16, tag="kT")
            qT_sb = P.tile([128, DC, QS], BF16, tag="qT")
            vplus = P.tile([128, SC128, 8 * 65], BF16, tag="vplus")
            vp4 = vplus.rearrange("p s (h c) -> p s h c", c=65)
            ebT = P.tile([128, SC128, QS], BF16, tag="ebT")
            ctxT_sb = P.tile([128, DC, QS], BF16, tag="ctxT")

            # ---- small input loads; wq/xqT lead the sync queue so the Q
            # projection unblocks first; scalar metadata rides one packed
            # [128,24] tensor (kpos | bq | bv) to cut per-DMA overheads ----
            w_sb = {}
            w_sb["wq"] = P.tile([128, DC, D], BF16, tag="wq", name="wq")
            nc.sync.dma_start(
                out=w_sb["wq"],
                in_=wqT.rearrange("(c p) s -> p c s", p=128))
            xqT_sb = P.tile([128, DC, QS], BF16, tag="xqT")
            nc.sync.dma_start(
                out=xqT_sb, in_=xqT.rearrange("(c p) s -> p c s", p=128))
            # metadata smalls ride the gpsimd queue HEAD so the one-hot /
            # combine chain unblocks early instead of landing ~21us behind
            # the big transfers
            misc_sb = P.tile([128, 24], F32, tag="misc")
            nc.gpsimd.dma_start(out=misc_sb, in_=misc)
            kposT_sb = misc_sb[:, 0:16]
            iota_col = misc_sb[0:NC, 0:1]
            cidkb = P.tile([NC, S], BF16, tag="cidkb")
            nc.gpsimd.dma_start(out=cidkb, in_=bcast_ap(cidk, NC))
            cidqb = P.tile([NC, QS], BF16, tag="cidqb")
            nc.gpsimd.dma_start(out=cidqb, in_=bcast_ap(cidq, NC))
            qposb = P.tile([128, QS], F16, tag="qposb")
            nc.sync.dma_start(out=qposb, in_=bcast_ap(qpos, 128))
            bo_row = P.tile([1, D], F32, tag="bo_row")
            nc.sync.dma_start(out=bo_row, in_=bo.rearrange("(a s) -> a s", a=1))
            if with_masks:
                kpmT_sb = P.tile([128, SC128], F32, tag="kpmT")
                nc.sync.dma_start(out=kpmT_sb, in_=kpmT)

            # ---- big input loads (gpsimd queue; issue order ~ arrival) ----
            def load_w(nm, ap_):
                w_sb[nm] = P.tile([128, DC, D], BF16, tag=nm, name=nm)
                nc.gpsimd.dma_start(
                    out=w_sb[nm], in_=ap_.rearrange("(c p) s -> p c s", p=128))

            load_w("wk", wkT)
            load_w("wv", wvT)
            xT_sb = P.tile([128, DC, S], BF16, tag="xT")

            def load_x_half(i):
                nc.gpsimd.dma_start(
                    out=xT_sb[:, :, i * 1024:(i + 1) * 1024],
                    in_=xT.rearrange("(c p) s -> p c s", p=128)[
                        :, :, i * 1024:(i + 1) * 1024])

            bias_in = {"ipa": [None] * 2, "asc": [None] * 2,
                       "msk": [None] * 2}

            def load_bias_g8(g8):
                specs = [("ipa", ipaT, IPA), ("asc", ascT, ASC)]
                if with_masks:
                    specs.append(("msk", mskT, EW))
                for nm, ap_, pool in specs:
                    t = pool.tile([128, 8, QS], BF16, tag="b" + nm, name=nm)
                    nc.gpsimd.dma_start(
                        out=t,
                        in_=ap_[g8 * 1024:(g8 + 1) * 1024, :].rearrange(
                            "(c p) s -> p c s", p=128))
                    bias_in[nm][g8] = t

            load_x_half(0)
            load_bias_g8(0)
            load_x_half(1)
            load_bias_g8(1)
            load_w("wo", woT)
            wo_sb = w_sb["wo"]

            bv_bf = P.tile([128, DC], BF16, tag="bv_bf")
            nc.vector.tensor_copy(bv_bf, misc_sb[:, 20:24])
            cvec = P.tile([1, D], BF16, tag="cvec")
            ones_row = P.tile([1, 128], BF16, tag="ones_row")
            nc.vector.memset(ones_row, 1.0)
            nc.vector.memset(vp4[:, :, :, 64:65], 1.0)

            # ---- one-hot concept encodings (DVE, tiny) ----
            ohk = P.tile([NC, S], BF16, tag="ohk")
            nc.vector.tensor_scalar(
                out=ohk, in0=cidkb, scalar1=iota_col, scalar2=None,
                op0=AL.is_equal)
            ohq = P.tile([NC, QS], BF16, tag="ohq")
            nc.vector.tensor_scalar(
                out=ohq, in0=cidqb, scalar1=iota_col, scalar2=0.5,
                op0=AL.is_equal, op1=AL.mult)

            # ---- concept-score matmuls; copied to SBUF right away so the
            # PSUM ring slots are not held hostage to the bias DMAs ----
            cs_sb = P.tile([128, SC128, QS], mybir.dt.float8e4,
                           tag="cs_sb")  # values in {0, 0.5}: exact in e4m3
            for p in range(8):
                cs = SPP.tile([128, 2, 512], F32, tag="sp", name="cs")
                for j in range(2):
                    kc = p * 2 + j
                    nc.tensor.matmul(
                        cs[:, j, :],
                        lhsT=ohk[:, kc * 128:(kc + 1) * 128],
                        rhs=ohq,
                        start=True, stop=True)
                dst = cs_sb[:, p * 2:(p + 1) * 2, :]
                if p % 2 == 0:
                    nc.vector.tensor_copy(dst, cs)
                else:
                    nc.scalar.copy(out=dst, in_=cs)

            # ---- combined bias -> exp, in [k, q] layout, per kc pair ----
            # xfall = cs*(qpos!=kpos) + ipa + asc (+ msk + kpm); eb = exp().
            def combine_pair(p):
                g8 = p // 4
                ipa_t = bias_in["ipa"][g8]
                asc_t = bias_in["asc"][g8]
                jj = (p % 4) * 2
                xf = XF.tile([128, 2, QS], BF16, tag="xf", name="xf")
                for j in range(2):
                    kc = p * 2 + j
                    # xf = (qpos != kpos) * cs  -- one fused op per chunk
                    nc.vector.scalar_tensor_tensor(
                        out=xf[:, j, :], in0=qposb,
                        scalar=kposT_sb[:, kc:kc + 1],
                        in1=cs_sb[:, p * 2 + j, :],
                        op0=AL.not_equal, op1=AL.mult)
                t1 = XF.tile([128, 2, QS], BF16, tag="t1", name="t1")
                nc.vector.tensor_tensor(
                    out=t1, in0=xf, in1=ipa_t[:, jj:jj + 2, :], op=AL.add)
                dst = XF.tile([128, 2, QS], BF16, tag="t2", name="t2")
                nc.vector.tensor_tensor(
                    out=dst, in0=t1, in1=asc_t[:, jj:jj + 2, :], op=AL.add)
                if with_masks:
                    dst2 = XF.tile([128, 2, QS], BF16, tag="t3", name="t3")
                    nc.vector.tensor_tensor(
                        out=dst2, in0=dst, in1=bias_in["msk"][g8][:, jj:jj + 2, :],
                        op=AL.add)
                    kcol = EW.tile([128, 2, QS], BF16, tag="kp", name="kp")
                    for j in range(2):
                        kc = p * 2 + j
                        nc.vector.tensor_scalar(
                            out=kcol[:, j, :], in0=dst2[:, j, :],
                            scalar1=kpmT_sb[:, kc:kc + 1], scalar2=None,
                            op0=AL.add)
                    dst = kcol
                # exp on Act
                nc.scalar.activation(
                    out=ebT[:, p * 2:(p + 1) * 2, :], in_=dst, func=AF.Exp)

            # ---- Q projection (+bq via Act identity) ----
            for ocp in range(2):
                ps = SPP.tile([128, 2, 512], F32, tag="sp", name="psq")
                for i in range(2):
                    oc = ocp * 2 + i
                    for dc in range(DC):
                        nc.tensor.matmul(
                            ps[:, i, :],
                            lhsT=w_sb["wq"][:, dc, oc * 128:(oc + 1) * 128],
                            rhs=xqT_sb[:, dc, :],
                            start=(dc == 0), stop=(dc == DC - 1))
                for i in range(2):
                    oc = ocp * 2 + i
                    nc.scalar.activation(
                        out=qT_sb[:, oc, :], in_=ps[:, i, :],
                        func=AF.Identity, bias=misc_sb[:, 16 + oc:17 + oc])

            # ---- attention main loop ----
            norm_state = {}

            def norm_step(step, hprev, cps_prev):
                ocp_, rbp = hprev // 2, (hprev % 2) * 64
                if step == 0:
                    rs_row = RW.tile([1, QS], F32, tag="rs_row",
                                     name="rs_row")
                    nc.vector.tensor_copy(rs_row, cps_prev[64:65, :])
                    rr = RW.tile([1, QS], F32, tag="rr", name="rr")
                    nc.vector.reciprocal_approx_fast(rr, rs_row)
                    norm_state["rr"] = rr
                elif step == 1:
                    rrb = RW.tile([64, QS], F32, tag="rrb", name="rrb")
                    nc.gpsimd.partition_broadcast(rrb, norm_state["rr"])
                    norm_state["rrb"] = rrb
                else:
                    nc.vector.tensor_tensor(
                        out=ctxT_sb[rbp:rbp + 64, ocp_, :],
                        in0=cps_prev[0:64, :], in1=norm_state["rrb"],
                        op=AL.mult)

            # ---- JIT emitters: K sc-block / V pair produced inside h0's
            # loop right before their first consumer, so no engine queue
            # entry waits on a DMA that lands later than its turn ----
            def emit_k_block(sc):
                for ocp in range(2):
                    ps = SPP.tile([128, 2, 512], F32, tag="sp", name="psk")
                    for i in range(2):
                        oc = ocp * 2 + i
                        for dc in range(DC):
                            nc.tensor.matmul(
                                ps[:, i, :],
                                lhsT=w_sb["wk"][:, dc,
                                                oc * 128:(oc + 1) * 128],
                                rhs=xT_sb[:, dc, sc * 512:(sc + 1) * 512],
                                start=(dc == 0), stop=(dc == DC - 1))
                    dst = kT_sb[:, ocp * 2:(ocp + 1) * 2,
                                sc * 512:(sc + 1) * 512]
                    if ocp == 0:
                        nc.vector.tensor_copy(dst, ps)
                    else:
                        nc.scalar.copy(out=dst, in_=ps)

            def emit_v_pair(scp):
                ps = SPP.tile([128, 2, 512], F32, tag="sp", name="psv")
                for i in range(2):
                    sc = scp * 2 + i
                    for dc in range(DC):
                        nc.tensor.matmul(
                            ps[:, i, :],
                            lhsT=xT_sb[:, dc, sc * 128:(sc + 1) * 128],
                            rhs=w_sb["wv"][:, dc, :],
                            start=(dc == 0), stop=(dc == DC - 1))
                vdst = vp4[:, scp * 2:(scp + 1) * 2, :, 0:64]
                vsrc = ps.rearrange("p i (h c) -> p i h c", c=64)
                if scp % 2 == 0:
                    nc.vector.tensor_copy(vdst, vsrc)
                else:
                    nc.scalar.copy(out=vdst, in_=vsrc)

            # attnV emission lags the scores/exp stream by ATTNV_LAG groups
            # so the in-order PE queue never stalls waiting for praw/pf of
            # the group it just produced (software pipelining).
            ATTNV_LAG = 3

            def emit_attnv(cps_t, h, g, pf):
                for j in range(2):
                    kc = g * 2 + j
                    nc.tensor.matmul(
                        cps_t,
                        lhsT=vplus[:, kc, h * 65:(h + 1) * 65],
                        rhs=pf[:, j, :],
                        start=(kc == 0), stop=(kc == SC128 - 1))

            pending = None
            attnv_q = []
            for h in range(H):
                oc, rb = h // 2, (h % 2) * 64
                cps_h = CP.tile([65, QS], F32, tag="ctx", name="ctx")
                for g in range(8):
                    if h == 0 and g % 2 == 0:
                        emit_k_block(g // 2)
                    ps = SPP.tile([128, 2, 512], F32, tag="sp", name="pss")
                    for j in range(2):
                        kc = g * 2 + j
                        nc.tensor.matmul(
                            ps[:, j, :],
                            lhsT=kT_sb[rb:rb + 64, oc,
                                       kc * 128:(kc + 1) * 128],
                            rhs=qT_sb[rb:rb + 64, oc, :],
                            start=True, stop=True)
                    praw = PRW.tile([128, 2, 512], BF16, tag="praw",
                                    name="praw")
                    nc.scalar.activation(out=praw, in_=ps, func=AF.Exp,
                                         scale=0.125)
                    if h == 0:
                        combine_pair(g)
                    pf = PW.tile([128, 2, 512], BF16, tag="pf", name="pf")
                    nc.vector.tensor_tensor(
                        out=pf, in0=praw,
                        in1=ebT[:, g * 2:(g + 1) * 2, :], op=AL.mult)
                    if h == 0:
                        emit_v_pair(g)
                    attnv_q.append((cps_h, h, g, pf))
                    # h0's attnVs defer a full head (their pf chain is
                    # DMA-gated; queuing them early would head-block the
                    # in-order PE queue and stall the score/exp stream),
                    # then drain gradually back to the steady lag
                    lag_now = 8 if h == 0 else ATTNV_LAG
                    pops = 0
                    while len(attnv_q) > lag_now and pops < 2:
                        emit_attnv(*attnv_q.pop(0))
                        pops += 1
                    if pending is not None and 4 <= g <= 6:
                        norm_step(g - 4, *pending)
                        if g == 6:
                            pending = None
                pending = (h, cps_h)
            while attnv_q:
                emit_attnv(*attnv_q.pop(0))

            # cvec = Wo @ bv + bo  (rank-1 epilogue row)
            cps = SPP.tile([128, 2, 512], F32, tag="sp", name="cps")
            for dc in range(DC):
                nc.tensor.matmul(cps[0:1, 0, :], lhsT=bv_bf[:, dc:dc + 1],
                                 rhs=wo_sb[:, dc, :],
                                 start=(dc == 0), stop=(dc == DC - 1))
            nc.vector.tensor_tensor(out=cvec, in0=cps[0:1, 0, :], in1=bo_row,
                                    op=AL.add)

            # ---- output projection; dc 0..2 (heads 0..5) for the first
            # three row-tiles overlap h7's tail before its norm lands ----
            poms = []
            for m in range(3):
                pom = SPP.tile([128, 2, 512], F32, tag="sp", name="pom")
                for dc in range(3):
                    nc.tensor.matmul(
                        pom[:, 0, :],
                        lhsT=ctxT_sb[:, dc, m * 128:(m + 1) * 128],
                        rhs=wo_sb[:, dc, :],
                        start=(dc == 0), stop=False)
                poms.append(pom)

            for step in range(3):
                norm_step(step, *pending)

            def finish_pom(pom, m, dcs):
                for dc in dcs:
                    nc.tensor.matmul(
                        pom[:, 0, :],
                        lhsT=ctxT_sb[:, dc, m * 128:(m + 1) * 128],
                        rhs=wo_sb[:, dc, :],
                        start=(dc == 0), stop=False)
                nc.tensor.matmul(pom[:, 0, :], lhsT=ones_row, rhs=cvec,
                                 start=False, stop=True)
                nc.vector.tensor_copy(o_sb[:, m, :], pom[:, 0, :])

            o_sb = P.tile([128, QT, 512], BF16, tag="o_sb")
            for m in range(3):
                finish_pom(poms[m], m, [3])
            pom3 = SPP.tile([128, 2, 512], F32, tag="sp", name="pom")
            finish_pom(pom3, 3, [0, 1, 2, 3])
            nc.sync.dma_start(
                out=out.rearrange("(m p) s -> p m s", p=128), in_=o_sb)

    nc.compile()
    return nc


def _prep_in_maps(inputs, with_masks):
    x = np.asarray(inputs["x"], np.float32)
    ipa = np.asarray(inputs["ipa_affinity_bias"], np.float32)
    asc = np.asarray(inputs["assoc_bias"], np.float32)
    msk = np.asarray(inputs["attention_mask"], np.float32)
    cid = np.asarray(inputs["concept_ids"])
    kpm = np.asarray(inputs["key_padding_mask"])

    wT = {nm: np.ascontiguousarray(
        np.asarray(inputs[nm], np.float32).T.astype(BF))
        for nm in ("Wq", "Wk", "Wv", "Wo")}
    bias = {nm: np.asarray(inputs[nm], np.float32)
            for nm in ("bq", "bv", "bo")}
    kpos = np.arange(S, dtype=np.float32)
    misc = np.zeros((128, 24), np.float32)
    misc[:, 0:16] = kpos.reshape(SC128, 128).T
    misc[:, 16:20] = bias["bq"].reshape(DC, 128).T
    misc[:, 20:24] = bias["bv"].reshape(DC, 128).T

    xT = [np.ascontiguousarray(x[b].T.astype(BF)) for b in range(B)]
    F8NP = ml_dtypes.float8_e4m3
    ipaT = [[np.ascontiguousarray(
        ipa[b, q0:q0 + QS].T.astype(F8NP)) for q0 in range(0, S, QS)]
        for b in range(B)]
    ascT = [[np.ascontiguousarray(
        asc[b, q0:q0 + QS].T.astype(F8NP)) for q0 in range(0, S, QS)]
        for b in range(B)]
    cidq_f = np.where(cid >= 0, cid, -1).astype(BF)
    cidk_f = np.where(cid >= 0, cid, -2).astype(BF)
    kpm_add = np.where(kpm, np.float32(-1e30), np.float32(0.0))

    in_maps = []
    for c in range(N_CORES):
        b, qi = c // 4, c % 4
        q0 = qi * QS
        m = {
            "xT": xT[b],
            "xqT": np.ascontiguousarray(xT[b][:, q0:q0 + QS]),
            "wqT": wT["Wq"], "wkT": wT["Wk"],
            "wvT": wT["Wv"], "woT": wT["Wo"],
            "bo": bias["bo"], "misc": misc,
            "ipaT": ipaT[b][qi],
            "ascT": ascT[b][qi],
            "cidq": np.ascontiguousarray(cidq_f[b, q0:q0 + QS]),
            "cidk": np.ascontiguousarray(cidk_f[b]),
            "qpos": (q0 + np.arange(QS)).astype(np.float16),
        }
        if with_masks:
            m["mskT"] = np.ascontiguousarray(
                msk[q0:q0 + QS].T.astype(BF))
            m["kpmT"] = np.ascontiguousarray(
                kpm_add[b].reshape(SC128, 128).T)
        in_maps.append(m)
    return in_maps


def run(inputs, trace=False):
    msk = np.asarray(inputs["attention_mask"])
    kpm = np.asarray(inputs["key_padding_mask"])
    with_masks = bool(msk.any() or kpm.any())
    if with_masks not in _COMPILED:
        _COMPILED[with_masks] = _build(with_masks)
    nc = _COMPILED[with_masks]
    in_maps = _prep_in_maps(inputs, with_masks)
    kw = {}
    if trace:
        kw = dict(trace=True, trace_cores=list(range(N_CORES)))
    res = bass_utils.run_bass_kernel_spmd(
        nc, in_maps, core_ids=list(range(N_CORES)), **kw)
    out = np.empty((B, S, D), np.float32)
    for c in range(N_CORES):
        b, q0 = c // 4, (c % 4) * QS
        out[b, q0:q0 + QS] = np.asarray(res.results[c]["out"],
                                        dtype=np.float32)
    return out, res


def kernel(**inputs) -> np.ndarray:
    out, _ = run(inputs)
    return out
